# revision 19
# baseline (speedup 1.0000x reference)
"""Householder reflection kernel for Trainium2 (8 NeuronCores, data-parallel).

Computes: v_new = v @ W.T + b
          z_new = z - 2 * v_new * (v_new . z) / ||v_new||^2

Full inputs: z [524288, 128] f32, v [524288, 128] f32, W [128, 128] f32, b [128] f32.
Sharding: batch split 8 ways; W/b replicated.

Strategy (fp16 data movement, fp32 accumulation), all row-major (rows on
partitions, features on the free axis), groups of 2048 rows (K=16 chunks):
  - Host: cast z to fp16; build vt = v.T in fp16 with rows permuted so that
    the z/out DMAs and the vt DMA are >=4KB-contiguous per partition and each
    matmul's stationary operand is a [feat, rows] chunk of vt.
    Row mapping: global row r = g*2048 + p*16 + k; vt column order
    g*2048 + k*128 + p.
  - PE: per 128-row chunk, psum[:,k,:] = vt_chunk.T @ W.T (fp16 in, f32 acc),
    bias added via a rank-1 ones*b4 matmul per PSUM bank.
  - ACT: copy psum -> vn (fp16 sbuf) per half-group; Square(vn) -> cmb[:,K:2K].
  - DVE (the bottleneck; everything at 2x fp16):
      pr = vn*z -> cmb[:,0:K]; one shared tree (hh, hh2, tensor_reduce)
      reduces dot and norm together; s = -2*dot*recip(norm);
      t = vn*s as ONE tensor_tensor with s pair-duplicated to fp16 [P,K,2]
      so the broadcast AP's innermost dim is [stride 1, count 2] (2x-eligible,
      unlike a plain stride-0 broadcast); zn = t + z pipelined one group
      behind so the in-order DVE queue never stalls on s.
  - Pool/GPSIMD: intentionally UNUSED. Its SBUF port is DVE's rd1: any
    concurrent Pool op roughly halves DVE 2-input throughput (measured ~2x
    tax on overlapped tensor_tensor ops, up to 7x in microbenchmarks).
"""

import sys

if "/opt/trn_rl_repo" not in sys.path:
    sys.path.insert(0, "/opt/trn_rl_repo")

import numpy as np

B = 524288
D = 128
NCORES = 8
ROWS_PER_CORE = B // NCORES          # 65536
P = 128                              # rows per chunk (psum partitions)
K = 16                               # chunks per group
ROWS_PER_GROUP = K * P               # 2048
GROUPS = ROWS_PER_CORE // ROWS_PER_GROUP  # 32
GT = B // ROWS_PER_GROUP             # 256 global groups

TRACE = False
TRACE_KW = {}
LAST = None
NACC = 0  # chunks whose norm goes via per-chunk ACT Square+accum (measured:
          # each costs 574ns on ACT vs 89ns saved on DVE -- keep at 0)

_compiled = None


def _build():
    import concourse.bacc as bacc
    import concourse.tile as tile
    from concourse import mybir

    nc = bacc.Bacc("TRN2")
    f16 = mybir.dt.float16
    f32 = mybir.dt.float32
    Alu = mybir.AluOpType
    Act = mybir.ActivationFunctionType

    z_d = nc.dram_tensor("z", [ROWS_PER_CORE, D], f16, kind="ExternalInput")
    vt_d = nc.dram_tensor("vt", [D, ROWS_PER_CORE], f16, kind="ExternalInput")
    wt_d = nc.dram_tensor("wt", [D, D], f16, kind="ExternalInput")
    b4_d = nc.dram_tensor("b4", [1, 4 * D], f16, kind="ExternalInput")
    out_d = nc.dram_tensor("z_new", [ROWS_PER_CORE, D], f16, kind="ExternalOutput")

    # DRAM views; all give >=4KB-contiguous per-partition DMA segments.
    # Loads are paired two groups per DMA to halve DGE/semaphore overhead.
    zv2 = z_d.rearrange("(g t p k) f -> g p t k f", t=2, p=P, k=K)
    ov = out_d.rearrange("(g p k) f -> g p k f", p=P, k=K)
    vv2 = vt_d.rearrange("f (g t k p) -> g f t k p", t=2, k=K, p=P)

    with tile.TileContext(nc) as tc:
        from contextlib import ExitStack

        with ExitStack() as ctx:
            singles = ctx.enter_context(tc.tile_pool(name="singles", bufs=1))
            inpool = ctx.enter_context(tc.tile_pool(name="inp", bufs=3))
            vnpool = ctx.enter_context(tc.tile_pool(name="vn", bufs=5))
            sqpool = ctx.enter_context(tc.tile_pool(name="sq", bufs=4))
            hpool = ctx.enter_context(tc.tile_pool(name="h", bufs=4))
            smpool = ctx.enter_context(tc.tile_pool(name="sm", bufs=6))
            znpool = ctx.enter_context(tc.tile_pool(name="zn", bufs=4))
            ppool = ctx.enter_context(tc.tile_pool(name="ps", bufs=2, space="PSUM"))

            # wt/b4 go on the scalar DMA queue so the sync queue starts on
            # the first vt/z pair immediately (6 serialized DMAs on one
            # queue cost ~2.2us fixed overhead each and delayed the first
            # matmul to ~12us).
            wt_sb = singles.tile([D, D], f16)
            nc.scalar.dma_start(out=wt_sb, in_=wt_d.ap())
            b4_sb = singles.tile([1, 4 * D], f16)
            nc.scalar.dma_start(out=b4_sb, in_=b4_d.ap())
            ones1 = singles.tile([1, D], f16)
            nc.vector.memset(ones1, 1.0)

            prev = None
            for g in range(GROUPS):
                if g % 2 == 0:
                    vt2 = inpool.tile([P, 2, K, P], f16, tag="vt")
                    z2 = inpool.tile([P, 2, K, D], f16, tag="z")
                    nc.sync.dma_start(out=vt2, in_=vv2[g // 2])
                    nc.sync.dma_start(out=z2, in_=zv2[g // 2])
                vt_t = vt2[:, g % 2]
                z_t = z2[:, g % 2]

                # Half-group PSUM tiles (2 banks, 2 bufs per tag): ACT drains
                # each half while PE fills the next.
                vn_t = vnpool.tile([P, K, D], f16, tag="vn")
                H = K // 2
                for h, ptag in ((0, "psa"), (1, "psb")):
                    ps = ppool.tile([P, H, D], f32, tag=ptag)
                    for bb in range(H // 4):
                        nc.tensor.matmul(
                            ps[:, 4 * bb : 4 * bb + 4, :],
                            lhsT=ones1,
                            rhs=b4_sb,
                            start=True,
                            stop=False,
                            skip_group_check=True,
                        )
                        for kk in range(4 * bb, 4 * bb + 4):
                            k = h * H + kk
                            nc.tensor.matmul(
                                ps[:, kk, :],
                                lhsT=vt_t[:, k, :],
                                rhs=wt_sb,
                                start=False,
                                stop=True,
                                skip_group_check=True,
                            )
                    nc.scalar.copy(out=vn_t[:, h * H : (h + 1) * H, :], in_=ps)

                # Combined tile: [:, 0:K] = pr = vn*z (DVE), [:, K:K+KH] =
                # vn^2 for the HIGH chunks only (ACT Square). The LOW NACC
                # chunks' norms are computed entirely on ACT via per-chunk
                # Square ops with accum_out (free-axis sum to [P,1]) --
                # shifting ~NACC*89ns/group of tree work from the DVE
                # bottleneck to ACT's spare capacity.
                KH = K - NACC
                cmb = sqpool.tile([P, K + KH, D], f16, tag="cmb")
                nc.vector.tensor_tensor(
                    out=cmb[:, 0:K, :], in0=vn_t, in1=z_t, op=Alu.mult
                )
                if KH > 0:
                    nc.scalar.activation(
                        out=cmb[:, K : K + KH, :], in_=vn_t[:, NACC:K, :], func=Act.Square
                    )
                if NACC > 0:
                    nacc = smpool.tile([P, NACC], f32, tag="nacc")
                    scr = smpool.tile([P, D], f16, tag="scr")
                    for k in range(NACC):
                        nc.scalar.activation(
                            out=scr,
                            in_=vn_t[:, k, :],
                            func=Act.Square,
                            accum_out=nacc[:, k : k + 1],
                        )
                hh = hpool.tile([P, K + KH, 64], f16, tag="hh")
                nc.vector.tensor_tensor(
                    out=hh, in0=cmb[:, :, 0:64], in1=cmb[:, :, 64:128], op=Alu.add
                )
                hh2 = hpool.tile([P, K + KH, 32], f16, tag="hh2")
                nc.vector.tensor_tensor(
                    out=hh2, in0=hh[:, :, 0:32], in1=hh[:, :, 32:64], op=Alu.add
                )
                nd_t = smpool.tile([P, K + KH], f32, tag="nd")
                nc.vector.tensor_reduce(
                    out=nd_t, in_=hh2, axis=mybir.AxisListType.X, op=Alu.add
                )

                # recip + s computed directly in pair-duplicated [P,K,2]
                # layout (stride-0 broadcast reads) so no separate
                # duplication op is needed before the 2x t multiply.
                rn2 = smpool.tile([P, K, 2], f32, tag="rn2")
                if NACC > 0:
                    nc.vector.reciprocal(
                        out=rn2[:, 0:NACC],
                        in_=nacc[:, 0:NACC].unsqueeze(2).broadcast_to([P, NACC, 2]),
                    )
                if KH > 0:
                    nc.vector.reciprocal(
                        out=rn2[:, NACC:K],
                        in_=nd_t[:, K : K + KH].unsqueeze(2).broadcast_to([P, KH, 2]),
                    )
                s16 = smpool.tile([P, K, 2], f16, tag="s16")
                nc.vector.scalar_tensor_tensor(
                    out=s16,
                    in0=nd_t[:, 0:K].unsqueeze(2).broadcast_to([P, K, 2]),
                    scalar=-2.0,
                    in1=rn2,
                    op0=Alu.mult,
                    op1=Alu.mult,
                )
                t_t = vnpool.tile([P, K, D], f16, tag="t")
                s_bc = s16.unsqueeze(2).broadcast_to([P, K, 64, 2])
                nc.vector.tensor_tensor(
                    out=t_t.rearrange("p k (f two) -> p k f two", two=2),
                    in0=vn_t.rearrange("p k (f two) -> p k f two", two=2),
                    in1=s_bc,
                    op=Alu.mult,
                )

                # Software-pipelined tail: finish z_new(g-1) = t(g-1)+z(g-1)
                # after issuing t(g) so the in-order DVE queue never stalls.
                if prev is not None:
                    pt, pz, pg = prev
                    zn_t = znpool.tile([P, K, D], f16, tag="zn")
                    nc.vector.tensor_tensor(out=zn_t, in0=pt, in1=pz, op=Alu.add)
                    nc.scalar.dma_start(out=ov[pg], in_=zn_t)
                prev = (t_t, z_t, g)

            pt, pz, pg = prev
            zn_t = znpool.tile([P, K, D], f16, tag="zn")
            nc.vector.tensor_tensor(out=zn_t, in0=pt, in1=pz, op=Alu.add)
            nc.scalar.dma_start(out=ov[pg], in_=zn_t)

    nc.compile()
    return nc


def _get_compiled():
    global _compiled
    if _compiled is None:
        _compiled = _build()
    return _compiled


def kernel(z, v, W, b):
    from concourse.bass_utils import run_bass_kernel_spmd

    nc = _get_compiled()

    z16 = np.ascontiguousarray(np.asarray(z), dtype=np.float16)
    v16 = np.asarray(v, dtype=np.float16)
    # vt[f, g*2048 + k*128 + p] = v[g*2048 + p*16 + k, f]
    vt = np.ascontiguousarray(
        v16.reshape(GT, P, K, D).transpose(3, 0, 2, 1).reshape(D, B)
    )
    wt = np.ascontiguousarray(np.asarray(W, dtype=np.float16).T)
    b4 = np.ascontiguousarray(
        np.tile(np.asarray(b, dtype=np.float16).reshape(1, D), (1, 4))
    )

    in_maps = []
    for c in range(NCORES):
        sl = slice(c * ROWS_PER_CORE, (c + 1) * ROWS_PER_CORE)
        in_maps.append(
            {
                "z": z16[sl],
                "vt": np.ascontiguousarray(vt[:, sl]),
                "wt": wt,
                "b4": b4,
            }
        )

    res = run_bass_kernel_spmd(
        nc, in_maps, core_ids=list(range(NCORES)), trace=TRACE, **TRACE_KW
    )
    global LAST
    LAST = res
    out16 = np.concatenate([res.results[c]["z_new"] for c in range(NCORES)], axis=0)
    return out16.astype(np.float32)


# revision 20
# speedup vs baseline: 1.1971x; 1.1971x over previous
"""Householder reflection kernel for Trainium2 (8 NeuronCores, data-parallel).

Computes: v_new = v @ W.T + b
          z_new = z - 2 * v_new * (v_new . z) / ||v_new||^2

Full inputs: z [524288, 128] f32, v [524288, 128] f32, W [128, 128] f32, b [128] f32.
Sharding: batch split 8 ways; W/b replicated.

Strategy (fp16 data movement, fp32 accumulation), all row-major (rows on
partitions, features on the free axis), groups of 2048 rows (K=16 chunks):
  - Host: cast z to fp16; build vt = v.T in fp16 with rows permuted so that
    the z/out DMAs and the vt DMA are >=4KB-contiguous per partition and each
    matmul's stationary operand is a [feat, rows] chunk of vt.
    Row mapping: global row r = g*2048 + p*16 + k; vt column order
    g*2048 + k*128 + p.
  - PE: per 128-row chunk, psum[:,k,:] = vt_chunk.T @ W.T (fp16 in, f32 acc),
    bias added via a rank-1 ones*b4 matmul per PSUM bank.
  - ACT: copy psum -> vn (fp16 sbuf) per half-group; Square(vn) -> cmb[:,K:2K].
  - DVE (the bottleneck; everything at 2x fp16):
      pr = vn*z -> cmb[:,0:K]; one shared tree (hh, hh2, tensor_reduce)
      reduces dot and norm together; s = -2*dot*recip(norm);
      t = vn*s as ONE tensor_tensor with s pair-duplicated to fp16 [P,K,2]
      so the broadcast AP's innermost dim is [stride 1, count 2] (2x-eligible,
      unlike a plain stride-0 broadcast); zn = t + z pipelined one group
      behind so the in-order DVE queue never stalls on s.
  - Pool/GPSIMD: intentionally UNUSED. Its SBUF port is DVE's rd1: any
    concurrent Pool op roughly halves DVE 2-input throughput (measured ~2x
    tax on overlapped tensor_tensor ops, up to 7x in microbenchmarks).
"""

import sys

if "/opt/trn_rl_repo" not in sys.path:
    sys.path.insert(0, "/opt/trn_rl_repo")

import numpy as np

B = 524288
D = 128
NCORES = 8
ROWS_PER_CORE = B // NCORES          # 65536
P = 128                              # rows per chunk (psum partitions)
K = 16                               # chunks per group
ROWS_PER_GROUP = K * P               # 2048
GROUPS = ROWS_PER_CORE // ROWS_PER_GROUP  # 32
GT = B // ROWS_PER_GROUP             # 256 global groups

TRACE = False
TRACE_KW = {}
LAST = None
NACC = 0  # chunks whose norm goes via per-chunk ACT Square+accum (measured:
          # each costs 574ns on ACT vs 89ns saved on DVE -- keep at 0)

_compiled = None


def _build():
    import concourse.bacc as bacc
    import concourse.tile as tile
    from concourse import mybir

    nc = bacc.Bacc("TRN2")
    f16 = mybir.dt.float16
    f32 = mybir.dt.float32
    Alu = mybir.AluOpType
    Act = mybir.ActivationFunctionType

    z_d = nc.dram_tensor("z", [ROWS_PER_CORE, D], f16, kind="ExternalInput")
    vt_d = nc.dram_tensor("vt", [D, ROWS_PER_CORE], f16, kind="ExternalInput")
    wt_d = nc.dram_tensor("wt", [D, D], f16, kind="ExternalInput")
    b4_d = nc.dram_tensor("b4", [1, 4 * D], f16, kind="ExternalInput")
    out_d = nc.dram_tensor("z_new", [ROWS_PER_CORE, D], f16, kind="ExternalOutput")

    # DRAM views; all give >=4KB-contiguous per-partition DMA segments.
    # Loads are paired two groups per DMA to halve DGE/semaphore overhead.
    zv2 = z_d.rearrange("(g t p k) f -> g p t k f", t=2, p=P, k=K)
    ov = out_d.rearrange("(g p k) f -> g p k f", p=P, k=K)
    vv2 = vt_d.rearrange("f (g t k p) -> g f t k p", t=2, k=K, p=P)

    with tile.TileContext(nc) as tc:
        from contextlib import ExitStack

        with ExitStack() as ctx:
            singles = ctx.enter_context(tc.tile_pool(name="singles", bufs=1))
            inpool = ctx.enter_context(tc.tile_pool(name="inp", bufs=3))
            vnpool = ctx.enter_context(tc.tile_pool(name="vn", bufs=5))
            sqpool = ctx.enter_context(tc.tile_pool(name="sq", bufs=4))
            hpool = ctx.enter_context(tc.tile_pool(name="h", bufs=4))
            smpool = ctx.enter_context(tc.tile_pool(name="sm", bufs=6))
            znpool = ctx.enter_context(tc.tile_pool(name="zn", bufs=4))
            ppool = ctx.enter_context(tc.tile_pool(name="ps", bufs=2, space="PSUM"))

            # wt/b4 go on the scalar DMA queue so the sync queue starts on
            # the first vt/z pair immediately (6 serialized DMAs on one
            # queue cost ~2.2us fixed overhead each and delayed the first
            # matmul to ~12us).
            wt_sb = singles.tile([D, D], f16)
            nc.scalar.dma_start(out=wt_sb, in_=wt_d.ap())
            b4_sb = singles.tile([1, 4 * D], f16)
            nc.scalar.dma_start(out=b4_sb, in_=b4_d.ap())
            ones1 = singles.tile([1, D], f16)
            nc.vector.memset(ones1, 1.0)

            prev = None
            for g in range(GROUPS):
                if g % 2 == 0:
                    vt2 = inpool.tile([P, 2, K, P], f16, tag="vt")
                    z2 = inpool.tile([P, 2, K, D], f16, tag="z")
                    nc.sync.dma_start(out=vt2, in_=vv2[g // 2])
                    nc.sync.dma_start(out=z2, in_=zv2[g // 2])
                vt_t = vt2[:, g % 2]
                z_t = z2[:, g % 2]

                # Half-group PSUM tiles (2 banks, 2 bufs per tag): ACT drains
                # each half while PE fills the next.
                vn_t = vnpool.tile([P, K, D], f16, tag="vn")
                H = K // 2
                for h, ptag in ((0, "psa"), (1, "psb")):
                    ps = ppool.tile([P, H, D], f32, tag=ptag)
                    for bb in range(H // 4):
                        nc.tensor.matmul(
                            ps[:, 4 * bb : 4 * bb + 4, :],
                            lhsT=ones1,
                            rhs=b4_sb,
                            start=True,
                            stop=False,
                            skip_group_check=True,
                        )
                        for kk in range(4 * bb, 4 * bb + 4):
                            k = h * H + kk
                            nc.tensor.matmul(
                                ps[:, kk, :],
                                lhsT=vt_t[:, k, :],
                                rhs=wt_sb,
                                start=False,
                                stop=True,
                                skip_group_check=True,
                            )
                    nc.scalar.copy(out=vn_t[:, h * H : (h + 1) * H, :], in_=ps)

                # Combined tile: [:, 0:K] = pr = vn*z (DVE), [:, K:K+KH] =
                # vn^2 for the HIGH chunks only (ACT Square). The LOW NACC
                # chunks' norms are computed entirely on ACT via per-chunk
                # Square ops with accum_out (free-axis sum to [P,1]) --
                # shifting ~NACC*89ns/group of tree work from the DVE
                # bottleneck to ACT's spare capacity.
                KH = K - NACC
                cmb = sqpool.tile([P, K + KH, D], f16, tag="cmb")
                nc.vector.tensor_tensor(
                    out=cmb[:, 0:K, :], in0=vn_t, in1=z_t, op=Alu.mult
                )
                if KH > 0:
                    nc.scalar.activation(
                        out=cmb[:, K : K + KH, :], in_=vn_t[:, NACC:K, :], func=Act.Square
                    )
                # NOTE: these two tiles are allocated even at NACC=0 -- they
                # shift smpool's SBUF layout, and removing them measured
                # reproducibly ~46us SLOWER (SBUF address-dependent port/bank
                # conflicts between concurrently-accessed tiles).
                nacc = smpool.tile([P, max(NACC, 1)], f32, tag="nacc")
                scr = smpool.tile([P, D], f16, tag="scr")
                for k in range(NACC):
                    nc.scalar.activation(
                        out=scr,
                        in_=vn_t[:, k, :],
                        func=Act.Square,
                        accum_out=nacc[:, k : k + 1],
                    )
                hh = hpool.tile([P, K + KH, 64], f16, tag="hh")
                nc.vector.tensor_tensor(
                    out=hh, in0=cmb[:, :, 0:64], in1=cmb[:, :, 64:128], op=Alu.add
                )
                hh2 = hpool.tile([P, K + KH, 32], f16, tag="hh2")
                nc.vector.tensor_tensor(
                    out=hh2, in0=hh[:, :, 0:32], in1=hh[:, :, 32:64], op=Alu.add
                )
                nd_t = smpool.tile([P, K + KH], f32, tag="nd")
                nc.vector.tensor_reduce(
                    out=nd_t, in_=hh2, axis=mybir.AxisListType.X, op=Alu.add
                )

                # recip + s computed directly in pair-duplicated [P,K,2]
                # layout (stride-0 broadcast reads) so no separate
                # duplication op is needed before the 2x t multiply.
                rn2 = smpool.tile([P, K, 2], f32, tag="rn2")
                if NACC > 0:
                    nc.vector.reciprocal(
                        out=rn2[:, 0:NACC],
                        in_=nacc[:, 0:NACC].unsqueeze(2).broadcast_to([P, NACC, 2]),
                    )
                if KH > 0:
                    nc.vector.reciprocal(
                        out=rn2[:, NACC:K],
                        in_=nd_t[:, K : K + KH].unsqueeze(2).broadcast_to([P, KH, 2]),
                    )
                s16 = smpool.tile([P, K, 2], f16, tag="s16")
                nc.vector.scalar_tensor_tensor(
                    out=s16,
                    in0=nd_t[:, 0:K].unsqueeze(2).broadcast_to([P, K, 2]),
                    scalar=-2.0,
                    in1=rn2,
                    op0=Alu.mult,
                    op1=Alu.mult,
                )
                t_t = vnpool.tile([P, K, D], f16, tag="t")
                s_bc = s16.unsqueeze(2).broadcast_to([P, K, 64, 2])
                nc.vector.tensor_tensor(
                    out=t_t.rearrange("p k (f two) -> p k f two", two=2),
                    in0=vn_t.rearrange("p k (f two) -> p k f two", two=2),
                    in1=s_bc,
                    op=Alu.mult,
                )

                # Software-pipelined tail: finish z_new(g-1) = t(g-1)+z(g-1)
                # after issuing t(g) so the in-order DVE queue never stalls.
                if prev is not None:
                    pt, pz, pg = prev
                    zn_t = znpool.tile([P, K, D], f16, tag="zn")
                    nc.vector.tensor_tensor(out=zn_t, in0=pt, in1=pz, op=Alu.add)
                    nc.scalar.dma_start(out=ov[pg], in_=zn_t)
                prev = (t_t, z_t, g)

            pt, pz, pg = prev
            zn_t = znpool.tile([P, K, D], f16, tag="zn")
            nc.vector.tensor_tensor(out=zn_t, in0=pt, in1=pz, op=Alu.add)
            nc.scalar.dma_start(out=ov[pg], in_=zn_t)

    nc.compile()
    return nc


def _get_compiled():
    global _compiled
    if _compiled is None:
        _compiled = _build()
    return _compiled


def kernel(z, v, W, b):
    from concourse.bass_utils import run_bass_kernel_spmd

    nc = _get_compiled()

    z16 = np.ascontiguousarray(np.asarray(z), dtype=np.float16)
    v16 = np.asarray(v, dtype=np.float16)
    # vt[f, g*2048 + k*128 + p] = v[g*2048 + p*16 + k, f]
    vt = np.ascontiguousarray(
        v16.reshape(GT, P, K, D).transpose(3, 0, 2, 1).reshape(D, B)
    )
    wt = np.ascontiguousarray(np.asarray(W, dtype=np.float16).T)
    b4 = np.ascontiguousarray(
        np.tile(np.asarray(b, dtype=np.float16).reshape(1, D), (1, 4))
    )

    in_maps = []
    for c in range(NCORES):
        sl = slice(c * ROWS_PER_CORE, (c + 1) * ROWS_PER_CORE)
        in_maps.append(
            {
                "z": z16[sl],
                "vt": np.ascontiguousarray(vt[:, sl]),
                "wt": wt,
                "b4": b4,
            }
        )

    res = run_bass_kernel_spmd(
        nc, in_maps, core_ids=list(range(NCORES)), trace=TRACE, **TRACE_KW
    )
    global LAST
    LAST = res
    out16 = np.concatenate([res.results[c]["z_new"] for c in range(NCORES)], axis=0)
    return out16.astype(np.float32)


# revision 21
# speedup vs baseline: 1.2061x; 1.0075x over previous
"""Householder reflection kernel for Trainium2 (8 NeuronCores, data-parallel).

Computes: v_new = v @ W.T + b
          z_new = z - 2 * v_new * (v_new . z) / ||v_new||^2

Full inputs: z [524288, 128] f32, v [524288, 128] f32, W [128, 128] f32, b [128] f32.
Sharding: batch split 8 ways; W/b replicated.

Strategy (fp16 data movement, fp32 accumulation), all row-major (rows on
partitions, features on the free axis), groups of 2048 rows (K=16 chunks):
  - Host: cast z to fp16; build vt = v.T in fp16 with rows permuted so that
    the z/out DMAs and the vt DMA are >=4KB-contiguous per partition and each
    matmul's stationary operand is a [feat, rows] chunk of vt.
    Row mapping: global row r = g*2048 + p*16 + k; vt column order
    g*2048 + k*128 + p.
  - PE: per 128-row chunk, psum[:,k,:] = vt_chunk.T @ W.T (fp16 in, f32 acc),
    bias added via a rank-1 ones*b4 matmul per PSUM bank.
  - ACT: copy psum -> vn (fp16 sbuf) per half-group; Square(vn) -> cmb[:,K:2K].
  - DVE (the bottleneck; everything at 2x fp16):
      pr = vn*z -> cmb[:,0:K]; one shared tree (hh, hh2, tensor_reduce)
      reduces dot and norm together; s = -2*dot*recip(norm);
      t = vn*s as ONE tensor_tensor with s pair-duplicated to fp16 [P,K,2]
      so the broadcast AP's innermost dim is [stride 1, count 2] (2x-eligible,
      unlike a plain stride-0 broadcast); zn = t + z pipelined one group
      behind so the in-order DVE queue never stalls on s.
  - Pool/GPSIMD: intentionally UNUSED. Its SBUF port is DVE's rd1: any
    concurrent Pool op roughly halves DVE 2-input throughput (measured ~2x
    tax on overlapped tensor_tensor ops, up to 7x in microbenchmarks).
"""

import sys

if "/opt/trn_rl_repo" not in sys.path:
    sys.path.insert(0, "/opt/trn_rl_repo")

import numpy as np

B = 524288
D = 128
NCORES = 8
ROWS_PER_CORE = B // NCORES          # 65536
P = 128                              # rows per chunk (psum partitions)
K = 16                               # chunks per group
ROWS_PER_GROUP = K * P               # 2048
GROUPS = ROWS_PER_CORE // ROWS_PER_GROUP  # 32
GT = B // ROWS_PER_GROUP             # 256 global groups

TRACE = False
TRACE_KW = {}
LAST = None
NACC = 0  # chunks whose norm goes via per-chunk ACT Square+accum (measured:
          # each costs 574ns on ACT vs 89ns saved on DVE -- keep at 0)

_compiled = None


def _build():
    import concourse.bacc as bacc
    import concourse.tile as tile
    from concourse import mybir

    nc = bacc.Bacc("TRN2")
    f16 = mybir.dt.float16
    f32 = mybir.dt.float32
    Alu = mybir.AluOpType
    Act = mybir.ActivationFunctionType

    z_d = nc.dram_tensor("z", [ROWS_PER_CORE, D], f16, kind="ExternalInput")
    vt_d = nc.dram_tensor("vt", [D, ROWS_PER_CORE], f16, kind="ExternalInput")
    wt_d = nc.dram_tensor("wt", [D, D], f16, kind="ExternalInput")
    b4_d = nc.dram_tensor("b4", [1, 4 * D], f16, kind="ExternalInput")
    out_d = nc.dram_tensor("z_new", [ROWS_PER_CORE, D], f16, kind="ExternalOutput")

    # DRAM views; all give >=4KB-contiguous per-partition DMA segments.
    # Loads are paired two groups per DMA to halve DGE/semaphore overhead.
    zv2 = z_d.rearrange("(g t p k) f -> g p t k f", t=2, p=P, k=K)
    ov = out_d.rearrange("(g p k) f -> g p k f", p=P, k=K)
    vv2 = vt_d.rearrange("f (g t k p) -> g f t k p", t=2, k=K, p=P)

    with tile.TileContext(nc) as tc:
        from contextlib import ExitStack

        with ExitStack() as ctx:
            singles = ctx.enter_context(tc.tile_pool(name="singles", bufs=1))
            inpool = ctx.enter_context(tc.tile_pool(name="inp", bufs=3))
            vnpool = ctx.enter_context(tc.tile_pool(name="vn", bufs=5))
            sqpool = ctx.enter_context(tc.tile_pool(name="sq", bufs=4))
            hpool = ctx.enter_context(tc.tile_pool(name="h", bufs=4))
            smpool = ctx.enter_context(tc.tile_pool(name="sm", bufs=6))
            znpool = ctx.enter_context(tc.tile_pool(name="zn", bufs=4))
            ppool = ctx.enter_context(tc.tile_pool(name="ps", bufs=2, space="PSUM"))

            # wt/b4 lead the sync queue: the scalar DMA queue spins up ~3us
            # later than the sync queue, so parking them there delays the
            # first matmul.
            wt_sb = singles.tile([D, D], f16)
            nc.sync.dma_start(out=wt_sb, in_=wt_d.ap())
            b4_sb = singles.tile([1, 4 * D], f16)
            nc.sync.dma_start(out=b4_sb, in_=b4_d.ap())
            ones1 = singles.tile([1, D], f16)
            nc.vector.memset(ones1, 1.0)

            prev = None
            for g in range(GROUPS):
                if g % 2 == 0:
                    vt2 = inpool.tile([P, 2, K, P], f16, tag="vt")
                    z2 = inpool.tile([P, 2, K, D], f16, tag="z")
                    nc.sync.dma_start(out=vt2, in_=vv2[g // 2])
                    nc.sync.dma_start(out=z2, in_=zv2[g // 2])
                vt_t = vt2[:, g % 2]
                z_t = z2[:, g % 2]

                # Half-group PSUM tiles (2 banks, 2 bufs per tag): ACT drains
                # each half while PE fills the next.
                vn_t = vnpool.tile([P, K, D], f16, tag="vn")
                H = K // 2
                for h, ptag in ((0, "psa"), (1, "psb")):
                    ps = ppool.tile([P, H, D], f32, tag=ptag)
                    for bb in range(H // 4):
                        nc.tensor.matmul(
                            ps[:, 4 * bb : 4 * bb + 4, :],
                            lhsT=ones1,
                            rhs=b4_sb,
                            start=True,
                            stop=False,
                            skip_group_check=True,
                        )
                        for kk in range(4 * bb, 4 * bb + 4):
                            k = h * H + kk
                            nc.tensor.matmul(
                                ps[:, kk, :],
                                lhsT=vt_t[:, k, :],
                                rhs=wt_sb,
                                start=False,
                                stop=True,
                                skip_group_check=True,
                            )
                    nc.scalar.copy(out=vn_t[:, h * H : (h + 1) * H, :], in_=ps)

                # Combined tile: [:, 0:K] = pr = vn*z (DVE), [:, K:K+KH] =
                # vn^2 for the HIGH chunks only (ACT Square). The LOW NACC
                # chunks' norms are computed entirely on ACT via per-chunk
                # Square ops with accum_out (free-axis sum to [P,1]) --
                # shifting ~NACC*89ns/group of tree work from the DVE
                # bottleneck to ACT's spare capacity.
                KH = K - NACC
                cmb = sqpool.tile([P, K + KH, D], f16, tag="cmb")
                nc.vector.tensor_tensor(
                    out=cmb[:, 0:K, :], in0=vn_t, in1=z_t, op=Alu.mult
                )
                if KH > 0:
                    nc.scalar.activation(
                        out=cmb[:, K : K + KH, :], in_=vn_t[:, NACC:K, :], func=Act.Square
                    )
                # NOTE: these two tiles are allocated even at NACC=0 -- they
                # shift smpool's SBUF layout, and removing them measured
                # reproducibly ~46us SLOWER (SBUF address-dependent port/bank
                # conflicts between concurrently-accessed tiles).
                nacc = smpool.tile([P, max(NACC, 1)], f32, tag="nacc")
                scr = smpool.tile([P, D], f16, tag="scr")
                for k in range(NACC):
                    nc.scalar.activation(
                        out=scr,
                        in_=vn_t[:, k, :],
                        func=Act.Square,
                        accum_out=nacc[:, k : k + 1],
                    )
                hh = hpool.tile([P, K + KH, 64], f16, tag="hh")
                nc.vector.tensor_tensor(
                    out=hh, in0=cmb[:, :, 0:64], in1=cmb[:, :, 64:128], op=Alu.add
                )
                hh2 = hpool.tile([P, K + KH, 32], f16, tag="hh2")
                nc.vector.tensor_tensor(
                    out=hh2, in0=hh[:, :, 0:32], in1=hh[:, :, 32:64], op=Alu.add
                )
                nd_t = smpool.tile([P, K + KH], f32, tag="nd")
                nc.vector.tensor_reduce(
                    out=nd_t, in_=hh2, axis=mybir.AxisListType.X, op=Alu.add
                )

                # recip + s computed directly in pair-duplicated [P,K,2]
                # layout (stride-0 broadcast reads) so no separate
                # duplication op is needed before the 2x t multiply.
                rn2 = smpool.tile([P, K, 2], f32, tag="rn2")
                if NACC > 0:
                    nc.vector.reciprocal(
                        out=rn2[:, 0:NACC],
                        in_=nacc[:, 0:NACC].unsqueeze(2).broadcast_to([P, NACC, 2]),
                    )
                if KH > 0:
                    nc.vector.reciprocal(
                        out=rn2[:, NACC:K],
                        in_=nd_t[:, K : K + KH].unsqueeze(2).broadcast_to([P, KH, 2]),
                    )
                s16 = smpool.tile([P, K, 2], f16, tag="s16")
                nc.vector.scalar_tensor_tensor(
                    out=s16,
                    in0=nd_t[:, 0:K].unsqueeze(2).broadcast_to([P, K, 2]),
                    scalar=-2.0,
                    in1=rn2,
                    op0=Alu.mult,
                    op1=Alu.mult,
                )
                t_t = vnpool.tile([P, K, D], f16, tag="t")
                s_bc = s16.unsqueeze(2).broadcast_to([P, K, 64, 2])
                nc.vector.tensor_tensor(
                    out=t_t.rearrange("p k (f two) -> p k f two", two=2),
                    in0=vn_t.rearrange("p k (f two) -> p k f two", two=2),
                    in1=s_bc,
                    op=Alu.mult,
                )

                # Software-pipelined tail: finish z_new(g-1) = t(g-1)+z(g-1)
                # after issuing t(g) so the in-order DVE queue never stalls.
                if prev is not None:
                    pt, pz, pg = prev
                    zn_t = znpool.tile([P, K, D], f16, tag="zn")
                    nc.vector.tensor_tensor(out=zn_t, in0=pt, in1=pz, op=Alu.add)
                    nc.scalar.dma_start(out=ov[pg], in_=zn_t)
                prev = (t_t, z_t, g)

            pt, pz, pg = prev
            zn_t = znpool.tile([P, K, D], f16, tag="zn")
            nc.vector.tensor_tensor(out=zn_t, in0=pt, in1=pz, op=Alu.add)
            nc.scalar.dma_start(out=ov[pg], in_=zn_t)

    nc.compile()
    return nc


def _get_compiled():
    global _compiled
    if _compiled is None:
        _compiled = _build()
    return _compiled


def kernel(z, v, W, b):
    from concourse.bass_utils import run_bass_kernel_spmd

    nc = _get_compiled()

    z16 = np.ascontiguousarray(np.asarray(z), dtype=np.float16)
    v16 = np.asarray(v, dtype=np.float16)
    # vt[f, g*2048 + k*128 + p] = v[g*2048 + p*16 + k, f]
    vt = np.ascontiguousarray(
        v16.reshape(GT, P, K, D).transpose(3, 0, 2, 1).reshape(D, B)
    )
    wt = np.ascontiguousarray(np.asarray(W, dtype=np.float16).T)
    b4 = np.ascontiguousarray(
        np.tile(np.asarray(b, dtype=np.float16).reshape(1, D), (1, 4))
    )

    in_maps = []
    for c in range(NCORES):
        sl = slice(c * ROWS_PER_CORE, (c + 1) * ROWS_PER_CORE)
        in_maps.append(
            {
                "z": z16[sl],
                "vt": np.ascontiguousarray(vt[:, sl]),
                "wt": wt,
                "b4": b4,
            }
        )

    res = run_bass_kernel_spmd(
        nc, in_maps, core_ids=list(range(NCORES)), trace=TRACE, **TRACE_KW
    )
    global LAST
    LAST = res
    out16 = np.concatenate([res.results[c]["z_new"] for c in range(NCORES)], axis=0)
    return out16.astype(np.float32)
